# revision 17
# baseline (speedup 1.0000x reference)
"""HBiLSTM Trainium2 kernel (v5).

Strategy (8 NeuronCores):
  - cores 0-3: forward LSTM + fwd highway half, 8 samples each
  - cores 4-7: backward LSTM on host-reversed input + bwd highway half
  All cores run the SAME SPMD program; direction is encoded purely in the
  per-core input data (weights + pre-reversed/pre-transposed x).

Device layout: gate/hidden dims on SBUF partitions, batch (8) on the free
dim.  Host does all transposes / reversal / concat / masking (untimed).

The recurrence is latency-bound: total time = 512 x per-step critical
path.  Single chain of all 8 samples:
  - gate tile order [i,f,g,o]; i/f/o rows pre-scaled 0.5 on host (tanh
    half-angle sigmoid), g rows 1.0.
  - TWO psum banks per step: ps_ifg (6 tiles) and ps_o (2 tiles).  Each
    gets its xg chunk via an identity matmul (start=True) that the Whh@h
    matmuls accumulate onto.  Only the 12 ifg Whh matmuls + tanh_ifg sit
    on the critical path; the 4 o-tile matmuls + tanh_o run in its shadow.
  - scaled states c^ = 2c, h^ = 2h:
        A   = (th_f + 1) * c^           # 2 sig_f * c^
        B   = (th_i + 1) * th_g         # 2 sig_i * g
        c^' = 0.5*A + B                 # = 2 c_new
        tau = tanh(0.5 * c^')           # ACT free scale
        h^' = (th_o + 1) * tau          # = 2 h_new
  - WAKEUP AVOIDANCE: an engine that blocks on a semaphore pays a ~140ns
    wakeup penalty; an op whose input is already available when the
    engine reaches it starts in ~0.  Dep-free scratch "filler" ops sized
    to the dependency gaps keep ACT and DVE busy until each critical
    op's input has landed, and the I-MM-o for step t+1 is artificially
    made to wait on tau(t) so the PE parks right before the Whh burst.

Phases:
  A: xg.T = Wp @ x.T + b  (10 tiles); bias-add/copy ops round-robin
     DVE/ACT; overlaps the first ~30 recurrence steps (fillers off).
  B: 512-step recurrence; highway chunks interleaved at 128-step
     boundaries (last chunk only 32 wide to shrink the tail).
  C: highway gate flow = g_pre + sig(g_pre) * (h^/2 - g_pre), DMA out.
"""

import numpy as np
import ml_dtypes

bf16 = ml_dtypes.bfloat16

B, T, DIN, H = 32, 512, 512, 256
NG = 4 * H          # 1024 gate rows per direction
NP = NG + H         # 1280 = gates + highway-half rows
BPC = 8             # samples per core
NCORES = 8
TOK = BPC * T       # tokens per core = 4096

# filler sizing and warmup start step.  Fillers must burn engine time with
# minimal SBUF traffic: ACT fillers are chains of small [128,16] copies,
# DVE fillers are reciprocal ops (~6 cycles/element, tiny footprint).
FA1K = 0            # ACT copy-lets between tho and tau
FA2K = 5            # ACT copy-lets between tau and next tanh_ifg
FD1N = 56           # DVE recip elems between c^ and h^   (~420ns)
FD2N = 123          # DVE recip elems between h^ and next A (~830ns)
WARM0 = 80          # first step with fillers (phase A overlap before)

_PROG_CACHE = {}


def _build_program(n_steps=T):
    import concourse.bacc as bacc
    import concourse.mybir as mybir
    import concourse.tile as tile
    from concourse.tile import add_dep_helper

    fp32 = mybir.dt.float32
    b16 = mybir.dt.bfloat16
    Tanh = mybir.ActivationFunctionType.Tanh
    Sigmoid = mybir.ActivationFunctionType.Sigmoid
    Identity = mybir.ActivationFunctionType.Identity
    Copy = mybir.ActivationFunctionType.Copy
    ADD = mybir.AluOpType.add
    MULT = mybir.AluOpType.mult
    SUB = mybir.AluOpType.subtract

    nc = bacc.Bacc(None)

    xt_d = nc.dram_tensor("xt", [DIN, TOK], b16, kind="ExternalInput")
    wpt_d = nc.dram_tensor("wpt", [DIN, NP], b16, kind="ExternalInput")
    whht_d = nc.dram_tensor("whht", [H, NG], b16, kind="ExternalInput")
    bias_d = nc.dram_tensor("bias", [NP], fp32, kind="ExternalInput")
    ident_d = nc.dram_tensor("ident", [128, 128], b16, kind="ExternalInput")
    out_d = nc.dram_tensor("out", [128, 2, T, BPC], fp32, kind="ExternalOutput")
    scrout_d = nc.dram_tensor("scrout", [128, 8], fp32, kind="ExternalOutput")

    KT_A = DIN // 128      # 4 contraction tiles in phase A
    MT_A = NP // 128       # 10 output tiles in phase A (8 xg + 2 gpre)
    NCH_A = TOK // 512     # 8 token chunks of 512
    GT = NG // 128         # 8 gate tiles in recurrence
    NIFG = 6               # i,f,g tiles (0..5); o tiles are 6,7
    KT_B = H // 128        # 2 contraction tiles in recurrence
    FB = BPC               # 8 samples, single chain
    GBI = NIFG * FB        # 48 = ifg cols
    GBO = (GT - NIFG) * FB # 16 = o cols
    KB = KT_B * FB         # 16 = hidden cols

    with tile.TileContext(nc) as tc:
      with (
          tc.tile_pool(name="persist", bufs=1) as pp,
          tc.tile_pool(name="psumB", bufs=2, space="PSUM") as psb,
          tc.tile_pool(name="phaseC", bufs=1) as pcl,
      ):
        gpre = pp.tile([128, 2, T, BPC], fp32, tag="gpre")      # 32KB/p
        bias_sb = pp.tile([128, MT_A], fp32, tag="bias")
        nc.sync.dma_start(bias_sb[:], bias_d.rearrange("(m p) -> p m", p=128))

        whh_sb = pp.tile([128, KT_B, NG], b16, tag="whh")
        nc.sync.dma_start(whh_sb[:], whht_d.rearrange("(k p) m -> p k m", p=128))

        ident_sb = pp.tile([128, 128], b16, tag="ident")
        nc.sync.dma_start(ident_sb[:], ident_d[:, :])

        # yh layout [128, T+1, KT_B*FB]: step slice [:, t, :] is flat 2D
        yh = pp.tile([128, n_steps + 1, KB], b16, tag="yh")
        cst = pp.tile([128, KB], fp32, tag="cst")
        nc.gpsimd.memset(yh[:, 0, :], 0.0)
        nc.gpsimd.memset(cst[:], 0.0)

        # filler scratch: ping-pong chains (each filler reads the
        # previous one's output so DCE cannot remove them; anchored by a
        # tiny DMA to scrout at the end)
        sA = [pp.tile([128, 16], fp32, tag=f"sA{i}", name=f"sA{i}")
              for i in range(2)]
        sD = [pp.tile([128, 128], fp32, tag=f"sD{i}", name=f"sD{i}")
              for i in range(2)]
        for i in range(2):
            nc.gpsimd.memset(sA[i][:], 1.0)
            nc.gpsimd.memset(sD[i][:], 1.0)
        fill_idx = {"a": 0, "d": 0}

        def act_fill(k, after):
            f = None
            for _ in range(k):
                i = fill_idx["a"]
                f = nc.scalar.activation(sA[1 - i][:], sA[i][:], Copy)
                fill_idx["a"] = 1 - i
                if after is not None:
                    add_dep_helper(f.ins, after.ins, reason="pin ACT filler")
                after = None
            return f

        def dve_fill(n, after):
            i = fill_idx["d"]
            f = nc.vector.reciprocal(sD[1 - i][:, 0:n], sD[i][:, 0:n])
            fill_idx["d"] = 1 - i
            if after is not None:
                add_dep_helper(f.ins, after.ins, reason="pin DVE filler")
            return f

        def highway_chunk(t0, t1):
            w = t1 - t0
            gp = gpre[:, :, t0:t1, :]
            tg = pcl.tile([128, 2, w, BPC], fp32, tag="tg_c")
            nc.scalar.activation(tg[:], gp, Sigmoid)
            yc = pcl.tile([128, 2, w, BPC], fp32, tag="y_c")
            for kk in range(KT_B):
                # yc = h^/2 - gpre   (3D APs: TensorScalarPtr limit)
                nc.vector.scalar_tensor_tensor(
                    yc[:, kk, :, :],
                    yh[:, t0 + 1 : t1 + 1, kk * FB : (kk + 1) * FB],
                    0.5,
                    gp[:, kk, :, :],
                    MULT, SUB,
                )
            fl = pcl.tile([128, 2, w, BPC], fp32, tag="fl_c")
            nc.vector.tensor_mul(fl[:], tg[:], yc[:])
            nc.vector.tensor_add(fl[:], fl[:], gp)
            nc.sync.dma_start(out_d[:, :, t0:t1, :], fl[:])

        with tc.tile_pool(name="pxg", bufs=1) as pxg:
            # xg: [:, t, 0:48] = ifg cols, [:, t, 48:64] = o cols -- both
            # flat contiguous identity-matmul rhs slices
            xgc = pxg.tile([128, T, GT * FB], b16, tag="xgc")  # 64KB/p

            # ---------------- Phase A: projections ----------------
            with (
                tc.tile_pool(name="phaseA", bufs=2) as pa,
                tc.tile_pool(name="psumA", bufs=2, space="PSUM") as psa,
            ):
                wp_sb = pa.tile([128, KT_A, NP], b16, tag="wp", bufs=1)
                nc.sync.dma_start(
                    wp_sb[:], wpt_d.rearrange("(k p) m -> p k m", p=128)
                )
                TBC = 512 // BPC   # 64 timesteps per chunk
                xgv = xgc[:, :, :].rearrange("p t (g b) -> p t g b", b=FB)
                vodd = 0
                for n in range(NCH_A):
                    xt_sb = pa.tile([128, KT_A, 512], b16, tag="xt")
                    nc.sync.dma_start(
                        xt_sb[:],
                        xt_d.rearrange("(k p) n -> p k n", p=128)[
                            :, :, n * 512 : (n + 1) * 512
                        ],
                    )
                    for m in range(MT_A):
                        ps = psa.tile([128, 512], fp32, tag="psA")
                        for k in range(KT_A):
                            nc.tensor.matmul(
                                ps[:],
                                wp_sb[:, k, m * 128 : (m + 1) * 128],
                                xt_sb[:, k, :],
                                start=(k == 0),
                                stop=(k == KT_A - 1),
                            )
                        tchunk = ps[:].rearrange("p (t b) -> p t b", b=BPC)
                        t0 = n * TBC
                        t1 = (n + 1) * TBC
                        if m < GT:
                            # round-robin DVE / ACT to keep phase A PE-bound
                            if vodd % 2 == 0:
                                nc.vector.tensor_scalar_add(
                                    xgv[:, t0:t1, m, :],
                                    tchunk,
                                    bias_sb[:, m : m + 1],
                                )
                            else:
                                nc.scalar.activation(
                                    xgv[:, t0:t1, m, :],
                                    tchunk,
                                    Identity,
                                    bias=bias_sb[:, m : m + 1],
                                )
                            vodd += 1
                        else:
                            nc.vector.tensor_scalar_add(
                                gpre[:, m - GT, t0:t1, :],
                                tchunk,
                                bias_sb[:, m : m + 1],
                            )

            # ---------------- Phase B: recurrence ----------------
            with tc.tile_pool(name="phaseB", bufs=4) as pb:
                prev_tau = None
                prev_fa2 = None
                prev_fd2 = None
                for t in range(n_steps):
                    fill = t >= WARM0
                    psi = psb.tile([128, GBI], fp32, tag="psI", name="psI")
                    pso = psb.tile([128, GBO], fp32, tag="psO", name="psO")
                    # xg(t) into both banks (clears them).  I-MM-ifg is
                    # dep-free and prefetches right after the previous
                    # burst; I-MM-o is artificially held on tau(t-1) so
                    # the PE parks just before this step's Whh burst
                    # (one sem wakeup instead of two).
                    imi = nc.tensor.matmul(
                        psi[:], ident_sb[:], xgc[:, t, 0:GBI],
                        start=True, stop=False,
                    )
                    nc.tensor.matmul(
                        pso[:], ident_sb[:], xgc[:, t, GBI : GBI + GBO],
                        start=True, stop=False,
                    )
                    # critical-path matmuls first: i,f,g tiles
                    for m in range(NIFG):
                        for k in range(KT_B):
                            nc.tensor.matmul(
                                psi[:, m * FB : (m + 1) * FB],
                                whh_sb[:, k, m * 128 : (m + 1) * 128],
                                yh[:, t, k * FB : (k + 1) * FB],
                                start=False,
                                stop=(m == NIFG - 1 and k == KT_B - 1),
                            )
                    # o tiles off the critical path
                    for m in range(NIFG, GT):
                        for k in range(KT_B):
                            nc.tensor.matmul(
                                pso[:, (m - NIFG) * FB : (m - NIFG + 1) * FB],
                                whh_sb[:, k, m * 128 : (m + 1) * 128],
                                yh[:, t, k * FB : (k + 1) * FB],
                                start=False,
                                stop=(m == GT - 1 and k == KT_B - 1),
                            )
                    th = pb.tile([128, GBI], fp32, tag="th", name="th")
                    thi = nc.scalar.activation(th[:], psi[:], Tanh)
                    if prev_fa2 is not None:
                        add_dep_helper(thi.ins, prev_fa2.ins,
                                       reason="tanh_ifg after fA2(t-1)")
                    tho = pb.tile([128, GBO], fp32, tag="tho", name="tho")
                    thoi = nc.scalar.activation(tho[:], pso[:], Tanh)
                    fa1 = act_fill(FA1K, thoi) if (fill and FA1K) else None
                    # A = (th_f + 1) * c^   (= 2 sig_f c^)
                    A = pb.tile([128, KB], fp32, tag="A", name="A")
                    ai = nc.vector.scalar_tensor_tensor(
                        A[:], th[:, 2 * FB : 4 * FB], 1.0, cst[:], ADD, MULT
                    )
                    if prev_fd2 is not None:
                        add_dep_helper(ai.ins, prev_fd2.ins,
                                       reason="A after fD2(t-1)")
                    # B = (th_i + 1) * th_g (= 2 sig_i g)
                    Bt = pb.tile([128, KB], fp32, tag="B", name="B")
                    nc.vector.scalar_tensor_tensor(
                        Bt[:], th[:, 0 : 2 * FB], 1.0, th[:, 4 * FB : 6 * FB],
                        ADD, MULT,
                    )
                    # c^' = 0.5*A + B  (= 2 c_new)
                    ci = nc.vector.scalar_tensor_tensor(
                        cst[:], A[:], 0.5, Bt[:], MULT, ADD
                    )
                    fd1 = dve_fill(FD1N, ci) if fill else None
                    # tau = tanh(c^' / 2) = tanh(c_new)
                    tau = pb.tile([128, KB], fp32, tag="tau", name="tau")
                    prev_tau = nc.scalar.activation(
                        tau[:], cst[:], Tanh, scale=0.5
                    )
                    if fa1 is not None:
                        add_dep_helper(prev_tau.ins, fa1.ins,
                                       reason="tau after fA1")
                    prev_fa2 = act_fill(FA2K, prev_tau) if fill else None
                    # h^' = (th_o + 1) * tau (= 2 h_new)
                    hi = nc.vector.scalar_tensor_tensor(
                        yh[:, t + 1, :], tho[:], 1.0, tau[:], ADD, MULT,
                    )
                    if fd1 is not None:
                        add_dep_helper(hi.ins, fd1.ins,
                                       reason="h^ after fD1")
                    prev_fd2 = dve_fill(FD2N, hi) if fill else None
                    # interleave highway chunks so they use DVE/ACT slack;
                    # the final (short) chunk is emitted after the loop
                    if (t + 1) in (128, 256, 384, 480):
                        highway_chunk(t + 1 - 128 if t + 1 != 480 else 384,
                                      t + 1)

        # ---------------- Phase C: last highway chunk ----------------
        highway_chunk(T - 32, T)
        # anchor the filler chains so DCE keeps them
        nc.sync.dma_start(scrout_d[:, 0:4], sA[fill_idx["a"]][:, 0:4])
        nc.sync.dma_start(scrout_d[:, 4:8], sD[fill_idx["d"]][:, 0:4])

    nc.compile()
    return nc


def _reverse_padded_np(x, lens):
    t = np.arange(T)
    idx = np.where(t[None, :] < lens[:, None], lens[:, None] - 1 - t[None, :], t[None, :])
    return np.take_along_axis(x, idx[:, :, None], axis=1), idx


def kernel(x, Wih_f, Whh_f, bih_f, bhh_f, Wih_b, Whh_b, bih_b, bhh_b, Wg, bg,
           x_lengths, **_unused):
    from concourse.bass_utils import run_bass_kernel_spmd

    x = np.asarray(x, dtype=np.float32)
    lens = np.asarray(x_lengths).astype(np.int64)

    xr, idx = _reverse_padded_np(x, lens)

    # tanh half-angle row scaling (torch gate order i,f,g,o):
    # i,f rows 0.5; g rows 1.0; o rows 0.5; highway rows 1.0
    rs = np.ones((NP, 1), dtype=np.float64)
    rs[0:512] = 0.5
    rs[768:1024] = 0.5

    def dir_weights(Wih, Whh, bih, bhh, wg_half, bg_half):
        Wp = np.concatenate([np.asarray(Wih), wg_half], axis=0)  # [1280, 512]
        Wp = Wp * rs
        wpt = np.ascontiguousarray(Wp.T).astype(bf16)            # [512, 1280]
        # Whh gets the row scaling AND a 0.5 for the h^ = 2h input
        Whh_s = np.asarray(Whh) * rs[0:NG] * 0.5
        whht = np.ascontiguousarray(Whh_s.T).astype(bf16)        # [256, 1024]
        bias = (np.asarray(bih) + np.asarray(bhh))
        bias = np.concatenate([bias, bg_half]) * rs[:, 0]
        return wpt, whht, bias.astype(np.float32)

    Wg = np.asarray(Wg); bg = np.asarray(bg)
    fw = dir_weights(Wih_f, Whh_f, bih_f, bhh_f, Wg[0:H], bg[0:H])
    bw = dir_weights(Wih_b, Whh_b, bih_b, bhh_b, Wg[H:2*H], bg[H:2*H])

    ident = np.eye(128, dtype=bf16)

    in_maps = []
    for c in range(NCORES):
        fwd = c < 4
        s0 = (c % 4) * BPC
        xsrc = x if fwd else xr
        xt = np.ascontiguousarray(
            xsrc[s0 : s0 + BPC].transpose(2, 1, 0).reshape(DIN, TOK)
        ).astype(bf16)
        wpt, whht, bias = fw if fwd else bw
        in_maps.append({"xt": xt, "wpt": wpt, "whht": whht, "bias": bias,
                        "ident": ident})

    if "prog" not in _PROG_CACHE:
        _PROG_CACHE["prog"] = _build_program()
    nc = _PROG_CACHE["prog"]
    _PROG_CACHE["last_inmaps"] = in_maps

    res = run_bass_kernel_spmd(nc, in_maps, core_ids=list(range(NCORES)))

    full = np.zeros((B, T, 2 * H), dtype=np.float32)
    for c in range(NCORES):
        arr = np.asarray(res.results[c]["out"], dtype=np.float32)  # [128,2,T,BPC]
        half = arr.transpose(3, 2, 1, 0).reshape(BPC, T, H)
        s0 = (c % 4) * BPC
        if c < 4:
            full[s0 : s0 + BPC, :, 0:H] = half
        else:
            # un-reverse within valid lengths
            half = np.take_along_axis(half, idx[s0 : s0 + BPC][:, :, None], axis=1)
            full[s0 : s0 + BPC, :, H : 2 * H] = half

    mask = (np.arange(T)[None, :] < lens[:, None])[:, :, None]
    full *= mask
    return full


# revision 18
# speedup vs baseline: 1.5383x; 1.5383x over previous
"""HBiLSTM Trainium2 kernel (v4).

Strategy (8 NeuronCores):
  - cores 0-3: forward LSTM + fwd highway half, 8 samples each
  - cores 4-7: backward LSTM on host-reversed input + bwd highway half
  All cores run the SAME SPMD program; direction is encoded purely in the
  per-core input data (weights + pre-reversed/pre-transposed x).

Device layout: gate/hidden dims on SBUF partitions, batch (8) on the free
dim.  Host does all transposes / reversal / concat / masking (untimed).

The recurrence is latency-bound: total time = 512 x per-step critical
path.  v4 minimizes that path with a single chain of all 8 samples:
  - gate tile order [i,f,g,o] (natural torch order needs no permutation
    beyond i,f,g | o splitting), i/f rows pre-scaled 0.5 (tanh half-angle
    sigmoid), g rows 1.0, o rows 0.5.
  - TWO psum banks per step: ps_ifg (6 tiles) and ps_o (2 tiles).  Each
    gets its xg chunk via an identity matmul (start=True) that the Whh@h
    matmuls accumulate onto.  The identity matmuls depend only on phase A
    so they prefetch into the next buffer during the previous step's
    elementwise work; only the 12 ifg Whh matmuls + tanh_ifg sit on the
    critical path, the 4 o-tile matmuls + tanh_o run in the shadow.
  - scaled states c^ = 2c, h^ = 2h:
        A   = (th_f + 1) * c^           # 2 sig_f * c^
        B   = (th_i + 1) * th_g         # 2 sig_i * g
        c^' = 0.5*A + B                 # = 2 c_new
        tau = tanh(0.5 * c^')           # ACT free scale
        h^' = (th_o + 1) * tau          # = 2 h_new
    (scalar_tensor_tensor fusions; 0.5 for h^ as matmul input folded into
    Whh on host; output 0.5 folded into phase C.)

Phases:
  A: xg.T = Wp @ x.T + b  (Wp = [Wih(scaled); Wg_half] -> 10 tiles);
     bias-add/copy ops round-robin DVE/ACT so phase A is PE-bound.
  B: 512-step recurrence; highway chunks emitted at 128-step boundaries
     fill DVE/ACT slack.
  C: highway gate flow = g_pre + sig(g_pre) * (h^/2 - g_pre), DMA out.
"""

import numpy as np
import ml_dtypes

bf16 = ml_dtypes.bfloat16

B, T, DIN, H = 32, 512, 512, 256
NG = 4 * H          # 1024 gate rows per direction
NP = NG + H         # 1280 = gates + highway-half rows
BPC = 8             # samples per core
NCORES = 8
TOK = BPC * T       # tokens per core = 4096

_PROG_CACHE = {}


def _build_program(n_steps=T):
    import concourse.bacc as bacc
    import concourse.mybir as mybir
    import concourse.tile as tile

    fp32 = mybir.dt.float32
    b16 = mybir.dt.bfloat16
    Tanh = mybir.ActivationFunctionType.Tanh
    Sigmoid = mybir.ActivationFunctionType.Sigmoid
    Identity = mybir.ActivationFunctionType.Identity
    ADD = mybir.AluOpType.add
    MULT = mybir.AluOpType.mult
    SUB = mybir.AluOpType.subtract

    nc = bacc.Bacc(None)

    xt_d = nc.dram_tensor("xt", [DIN, TOK], b16, kind="ExternalInput")
    wpt_d = nc.dram_tensor("wpt", [DIN, NP], b16, kind="ExternalInput")
    whht_d = nc.dram_tensor("whht", [H, NG], b16, kind="ExternalInput")
    bias_d = nc.dram_tensor("bias", [NP], fp32, kind="ExternalInput")
    ident_d = nc.dram_tensor("ident", [128, 128], b16, kind="ExternalInput")
    out_d = nc.dram_tensor("out", [128, 2, T, BPC], fp32, kind="ExternalOutput")

    KT_A = DIN // 128      # 4 contraction tiles in phase A
    MT_A = NP // 128       # 10 output tiles in phase A (8 xg + 2 gpre)
    NCH_A = TOK // 512     # 8 token chunks of 512
    GT = NG // 128         # 8 gate tiles in recurrence
    NIFG = 6               # i,f,g tiles (0..5); o tiles are 6,7
    KT_B = H // 128        # 2 contraction tiles in recurrence
    FB = BPC               # 8 samples, single chain
    GBI = NIFG * FB        # 48 = ifg cols
    GBO = (GT - NIFG) * FB # 16 = o cols
    KB = KT_B * FB         # 16 = hidden cols

    with tile.TileContext(nc) as tc:
      with (
          tc.tile_pool(name="persist", bufs=1) as pp,
          tc.tile_pool(name="psumB", bufs=2, space="PSUM") as psb,
          tc.tile_pool(name="phaseC", bufs=2) as pcl,
      ):
        gpre = pp.tile([128, 2, T, BPC], fp32, tag="gpre")      # 32KB/p
        bias_sb = pp.tile([128, MT_A], fp32, tag="bias")
        nc.sync.dma_start(bias_sb[:], bias_d.rearrange("(m p) -> p m", p=128))

        whh_sb = pp.tile([128, KT_B, NG], b16, tag="whh")
        nc.sync.dma_start(whh_sb[:], whht_d.rearrange("(k p) m -> p k m", p=128))

        ident_sb = pp.tile([128, 128], b16, tag="ident")
        nc.sync.dma_start(ident_sb[:], ident_d[:, :])

        # yh layout [128, T+1, KT_B*FB]: step slice [:, t, :] is flat 2D
        yh = pp.tile([128, n_steps + 1, KB], b16, tag="yh")
        cst = pp.tile([128, KB], fp32, tag="cst")
        nc.gpsimd.memset(yh[:, 0, :], 0.0)
        nc.gpsimd.memset(cst[:], 0.0)

        TCC = 128  # phase C chunk (timesteps)

        def highway_chunk(cch):
            t0, t1 = cch * TCC, (cch + 1) * TCC
            gp = gpre[:, :, t0:t1, :]
            tg = pcl.tile([128, 2, TCC, BPC], fp32, tag="tg_c")
            nc.scalar.activation(tg[:], gp, Sigmoid)
            yc = pcl.tile([128, 2, TCC, BPC], fp32, tag="y_c")
            for kk in range(KT_B):
                # yc = h^/2 - gpre   (3D APs: TensorScalarPtr limit)
                nc.vector.scalar_tensor_tensor(
                    yc[:, kk, :, :],
                    yh[:, t0 + 1 : t1 + 1, kk * FB : (kk + 1) * FB],
                    0.5,
                    gp[:, kk, :, :],
                    MULT, SUB,
                )
            fl = pcl.tile([128, 2, TCC, BPC], fp32, tag="fl_c")
            nc.vector.tensor_mul(fl[:], tg[:], yc[:])
            nc.vector.tensor_add(fl[:], fl[:], gp)
            nc.sync.dma_start(out_d[:, :, t0:t1, :], fl[:])

        with tc.tile_pool(name="pxg", bufs=1) as pxg:
            # xg: [:, t, 0:48] = ifg cols, [:, t, 48:64] = o cols -- both
            # flat contiguous identity-matmul rhs slices
            xgc = pxg.tile([128, T, GT * FB], b16, tag="xgc")  # 64KB/p

            # ---------------- Phase A: projections ----------------
            with (
                tc.tile_pool(name="phaseA", bufs=2) as pa,
                tc.tile_pool(name="psumA", bufs=2, space="PSUM") as psa,
            ):
                wp_sb = pa.tile([128, KT_A, NP], b16, tag="wp", bufs=1)
                nc.sync.dma_start(
                    wp_sb[:], wpt_d.rearrange("(k p) m -> p k m", p=128)
                )
                TBC = 512 // BPC   # 64 timesteps per chunk
                xgv = xgc[:, :, :].rearrange("p t (g b) -> p t g b", b=FB)
                vodd = 0
                for n in range(NCH_A):
                    xt_sb = pa.tile([128, KT_A, 512], b16, tag="xt")
                    nc.sync.dma_start(
                        xt_sb[:],
                        xt_d.rearrange("(k p) n -> p k n", p=128)[
                            :, :, n * 512 : (n + 1) * 512
                        ],
                    )
                    for m in range(MT_A):
                        ps = psa.tile([128, 512], fp32, tag="psA")
                        for k in range(KT_A):
                            nc.tensor.matmul(
                                ps[:],
                                wp_sb[:, k, m * 128 : (m + 1) * 128],
                                xt_sb[:, k, :],
                                start=(k == 0),
                                stop=(k == KT_A - 1),
                            )
                        tchunk = ps[:].rearrange("p (t b) -> p t b", b=BPC)
                        t0 = n * TBC
                        t1 = (n + 1) * TBC
                        if m < GT:
                            # round-robin DVE / ACT to keep phase A PE-bound
                            if vodd % 2 == 0:
                                nc.vector.tensor_scalar_add(
                                    xgv[:, t0:t1, m, :],
                                    tchunk,
                                    bias_sb[:, m : m + 1],
                                )
                            else:
                                nc.scalar.activation(
                                    xgv[:, t0:t1, m, :],
                                    tchunk,
                                    Identity,
                                    bias=bias_sb[:, m : m + 1],
                                )
                            vodd += 1
                        else:
                            nc.vector.tensor_scalar_add(
                                gpre[:, m - GT, t0:t1, :],
                                tchunk,
                                bias_sb[:, m : m + 1],
                            )

            # ---------------- Phase B: recurrence ----------------
            with tc.tile_pool(name="phaseB", bufs=4) as pb:
                for t in range(n_steps):
                    psi = psb.tile([128, GBI], fp32, tag="psI", name="psI")
                    pso = psb.tile([128, GBO], fp32, tag="psO", name="psO")
                    # xg(t) into both banks (clears them); no dependency on
                    # h so these prefetch during the previous step's
                    # elementwise tail
                    nc.tensor.matmul(
                        psi[:], ident_sb[:], xgc[:, t, 0:GBI],
                        start=True, stop=False,
                    )
                    nc.tensor.matmul(
                        pso[:], ident_sb[:], xgc[:, t, GBI : GBI + GBO],
                        start=True, stop=False,
                    )
                    # critical-path matmuls first: i,f,g tiles
                    for m in range(NIFG):
                        for k in range(KT_B):
                            nc.tensor.matmul(
                                psi[:, m * FB : (m + 1) * FB],
                                whh_sb[:, k, m * 128 : (m + 1) * 128],
                                yh[:, t, k * FB : (k + 1) * FB],
                                start=False,
                                stop=(m == NIFG - 1 and k == KT_B - 1),
                            )
                    # o tiles off the critical path
                    for m in range(NIFG, GT):
                        for k in range(KT_B):
                            nc.tensor.matmul(
                                pso[:, (m - NIFG) * FB : (m - NIFG + 1) * FB],
                                whh_sb[:, k, m * 128 : (m + 1) * 128],
                                yh[:, t, k * FB : (k + 1) * FB],
                                start=False,
                                stop=(m == GT - 1 and k == KT_B - 1),
                            )
                    th = pb.tile([128, GBI], fp32, tag="th", name="th")
                    nc.scalar.activation(th[:], psi[:], Tanh)
                    tho = pb.tile([128, GBO], fp32, tag="tho", name="tho")
                    nc.scalar.activation(tho[:], pso[:], Tanh)
                    # A = (th_f + 1) * c^   (= 2 sig_f c^)
                    A = pb.tile([128, KB], fp32, tag="A", name="A")
                    nc.vector.scalar_tensor_tensor(
                        A[:], th[:, 2 * FB : 4 * FB], 1.0, cst[:], ADD, MULT
                    )
                    # B = (th_i + 1) * th_g (= 2 sig_i g)
                    Bt = pb.tile([128, KB], fp32, tag="B", name="B")
                    nc.vector.scalar_tensor_tensor(
                        Bt[:], th[:, 0 : 2 * FB], 1.0, th[:, 4 * FB : 6 * FB],
                        ADD, MULT,
                    )
                    # c^' = 0.5*A + B  (= 2 c_new)
                    nc.vector.scalar_tensor_tensor(
                        cst[:], A[:], 0.5, Bt[:], MULT, ADD
                    )
                    # tau = tanh(c^' / 2) = tanh(c_new)
                    tau = pb.tile([128, KB], fp32, tag="tau", name="tau")
                    nc.scalar.activation(tau[:], cst[:], Tanh, scale=0.5)
                    # h^' = (th_o + 1) * tau (= 2 h_new)
                    nc.vector.scalar_tensor_tensor(
                        yh[:, t + 1, :], tho[:], 1.0, tau[:], ADD, MULT,
                    )
                    # interleave highway chunks so they use DVE/ACT slack
                    if (t + 1) % TCC == 0 and (t + 1) < n_steps:
                        highway_chunk((t + 1) // TCC - 1)

        # ---------------- Phase C: last highway chunk ----------------
        highway_chunk(T // TCC - 1)

    nc.compile()
    return nc


def _reverse_padded_np(x, lens):
    t = np.arange(T)
    idx = np.where(t[None, :] < lens[:, None], lens[:, None] - 1 - t[None, :], t[None, :])
    return np.take_along_axis(x, idx[:, :, None], axis=1), idx


def kernel(x, Wih_f, Whh_f, bih_f, bhh_f, Wih_b, Whh_b, bih_b, bhh_b, Wg, bg,
           x_lengths, **_unused):
    from concourse.bass_utils import run_bass_kernel_spmd

    x = np.asarray(x, dtype=np.float32)
    lens = np.asarray(x_lengths).astype(np.int64)

    xr, idx = _reverse_padded_np(x, lens)

    # tanh half-angle row scaling (torch gate order i,f,g,o):
    # i,f rows 0.5; g rows 1.0; o rows 0.5; highway rows 1.0
    rs = np.ones((NP, 1), dtype=np.float64)
    rs[0:512] = 0.5
    rs[768:1024] = 0.5

    def dir_weights(Wih, Whh, bih, bhh, wg_half, bg_half):
        Wp = np.concatenate([np.asarray(Wih), wg_half], axis=0)  # [1280, 512]
        Wp = Wp * rs
        wpt = np.ascontiguousarray(Wp.T).astype(bf16)            # [512, 1280]
        # Whh gets the row scaling AND a 0.5 for the h^ = 2h input
        Whh_s = np.asarray(Whh) * rs[0:NG] * 0.5
        whht = np.ascontiguousarray(Whh_s.T).astype(bf16)        # [256, 1024]
        bias = (np.asarray(bih) + np.asarray(bhh))
        bias = np.concatenate([bias, bg_half]) * rs[:, 0]
        return wpt, whht, bias.astype(np.float32)

    Wg = np.asarray(Wg); bg = np.asarray(bg)
    fw = dir_weights(Wih_f, Whh_f, bih_f, bhh_f, Wg[0:H], bg[0:H])
    bw = dir_weights(Wih_b, Whh_b, bih_b, bhh_b, Wg[H:2*H], bg[H:2*H])

    ident = np.eye(128, dtype=bf16)

    in_maps = []
    for c in range(NCORES):
        fwd = c < 4
        s0 = (c % 4) * BPC
        xsrc = x if fwd else xr
        xt = np.ascontiguousarray(
            xsrc[s0 : s0 + BPC].transpose(2, 1, 0).reshape(DIN, TOK)
        ).astype(bf16)
        wpt, whht, bias = fw if fwd else bw
        in_maps.append({"xt": xt, "wpt": wpt, "whht": whht, "bias": bias,
                        "ident": ident})

    if "prog" not in _PROG_CACHE:
        _PROG_CACHE["prog"] = _build_program()
    nc = _PROG_CACHE["prog"]
    _PROG_CACHE["last_inmaps"] = in_maps

    res = run_bass_kernel_spmd(nc, in_maps, core_ids=list(range(NCORES)))

    full = np.zeros((B, T, 2 * H), dtype=np.float32)
    for c in range(NCORES):
        arr = np.asarray(res.results[c]["out"], dtype=np.float32)  # [128,2,T,BPC]
        half = arr.transpose(3, 2, 1, 0).reshape(BPC, T, H)
        s0 = (c % 4) * BPC
        if c < 4:
            full[s0 : s0 + BPC, :, 0:H] = half
        else:
            # un-reverse within valid lengths
            half = np.take_along_axis(half, idx[s0 : s0 + BPC][:, :, None], axis=1)
            full[s0 : s0 + BPC, :, H : 2 * H] = half

    mask = (np.arange(T)[None, :] < lens[:, None])[:, :, None]
    full *= mask
    return full


# revision 19
# speedup vs baseline: 1.5405x; 1.0015x over previous
"""HBiLSTM Trainium2 kernel (v4).

Strategy (8 NeuronCores):
  - cores 0-3: forward LSTM + fwd highway half, 8 samples each
  - cores 4-7: backward LSTM on host-reversed input + bwd highway half
  All cores run the SAME SPMD program; direction is encoded purely in the
  per-core input data (weights + pre-reversed/pre-transposed x).

Device layout: gate/hidden dims on SBUF partitions, batch (8) on the free
dim.  Host does all transposes / reversal / concat / masking (untimed).

The recurrence is latency-bound: total time = 512 x per-step critical
path.  v4 minimizes that path with a single chain of all 8 samples:
  - gate tile order [i,f,g,o] (natural torch order needs no permutation
    beyond i,f,g | o splitting), i/f rows pre-scaled 0.5 (tanh half-angle
    sigmoid), g rows 1.0, o rows 0.5.
  - TWO psum banks per step: ps_ifg (6 tiles) and ps_o (2 tiles).  Each
    gets its xg chunk via an identity matmul (start=True) that the Whh@h
    matmuls accumulate onto.  The identity matmuls depend only on phase A
    so they prefetch into the next buffer during the previous step's
    elementwise work; only the 12 ifg Whh matmuls + tanh_ifg sit on the
    critical path, the 4 o-tile matmuls + tanh_o run in the shadow.
  - scaled states c^ = 2c, h^ = 2h:
        A   = (th_f + 1) * c^           # 2 sig_f * c^
        B   = (th_i + 1) * th_g         # 2 sig_i * g
        c^' = 0.5*A + B                 # = 2 c_new
        tau = tanh(0.5 * c^')           # ACT free scale
        h^' = (th_o + 1) * tau          # = 2 h_new
    (scalar_tensor_tensor fusions; 0.5 for h^ as matmul input folded into
    Whh on host; output 0.5 folded into phase C.)

Phases:
  A: xg.T = Wp @ x.T + b  (Wp = [Wih(scaled); Wg_half] -> 10 tiles);
     bias-add/copy ops round-robin DVE/ACT so phase A is PE-bound.
  B: 512-step recurrence; highway chunks emitted at 128-step boundaries
     fill DVE/ACT slack.
  C: highway gate flow = g_pre + sig(g_pre) * (h^/2 - g_pre), DMA out.
"""

import numpy as np
import ml_dtypes

bf16 = ml_dtypes.bfloat16

B, T, DIN, H = 32, 512, 512, 256
NG = 4 * H          # 1024 gate rows per direction
NP = NG + H         # 1280 = gates + highway-half rows
BPC = 8             # samples per core
NCORES = 8
TOK = BPC * T       # tokens per core = 4096

_PROG_CACHE = {}


def _build_program(n_steps=T):
    import concourse.bacc as bacc
    import concourse.mybir as mybir
    import concourse.tile as tile

    fp32 = mybir.dt.float32
    b16 = mybir.dt.bfloat16
    Tanh = mybir.ActivationFunctionType.Tanh
    Sigmoid = mybir.ActivationFunctionType.Sigmoid
    Identity = mybir.ActivationFunctionType.Identity
    ADD = mybir.AluOpType.add
    MULT = mybir.AluOpType.mult
    SUB = mybir.AluOpType.subtract

    nc = bacc.Bacc(None)

    xt_d = nc.dram_tensor("xt", [DIN, TOK], b16, kind="ExternalInput")
    wpt_d = nc.dram_tensor("wpt", [DIN, NP], b16, kind="ExternalInput")
    whht_d = nc.dram_tensor("whht", [H, NG], b16, kind="ExternalInput")
    bias_d = nc.dram_tensor("bias", [NP], fp32, kind="ExternalInput")
    ident_d = nc.dram_tensor("ident", [128, 128], b16, kind="ExternalInput")
    out_d = nc.dram_tensor("out", [128, 2, T, BPC], fp32, kind="ExternalOutput")

    KT_A = DIN // 128      # 4 contraction tiles in phase A
    MT_A = NP // 128       # 10 output tiles in phase A (8 xg + 2 gpre)
    NCH_A = TOK // 512     # 8 token chunks of 512
    GT = NG // 128         # 8 gate tiles in recurrence
    NIFG = 6               # i,f,g tiles (0..5); o tiles are 6,7
    KT_B = H // 128        # 2 contraction tiles in recurrence
    FB = BPC               # 8 samples, single chain
    GBI = NIFG * FB        # 48 = ifg cols
    GBO = (GT - NIFG) * FB # 16 = o cols
    KB = KT_B * FB         # 16 = hidden cols

    with tile.TileContext(nc) as tc:
      with (
          tc.tile_pool(name="persist", bufs=1) as pp,
          tc.tile_pool(name="psumB", bufs=2, space="PSUM") as psb,
          tc.tile_pool(name="phaseC", bufs=2) as pcl,
      ):
        gpre = pp.tile([128, 2, T, BPC], fp32, tag="gpre")      # 32KB/p
        bias_sb = pp.tile([128, MT_A], fp32, tag="bias")
        nc.sync.dma_start(bias_sb[:], bias_d.rearrange("(m p) -> p m", p=128))

        whh_sb = pp.tile([128, KT_B, NG], b16, tag="whh")
        nc.sync.dma_start(whh_sb[:], whht_d.rearrange("(k p) m -> p k m", p=128))

        ident_sb = pp.tile([128, 128], b16, tag="ident")
        nc.sync.dma_start(ident_sb[:], ident_d[:, :])

        # yh layout [128, T+1, KT_B*FB]: step slice [:, t, :] is flat 2D
        yh = pp.tile([128, n_steps + 1, KB], b16, tag="yh")
        cst = pp.tile([128, KB], fp32, tag="cst")
        nc.gpsimd.memset(yh[:, 0, :], 0.0)
        nc.gpsimd.memset(cst[:], 0.0)

        TCC = 128  # phase C chunk (timesteps)

        def highway_chunk(cch):
            t0, t1 = cch * TCC, (cch + 1) * TCC
            gp = gpre[:, :, t0:t1, :]
            tg = pcl.tile([128, 2, TCC, BPC], fp32, tag="tg_c")
            nc.scalar.activation(tg[:], gp, Sigmoid)
            yc = pcl.tile([128, 2, TCC, BPC], fp32, tag="y_c")
            for kk in range(KT_B):
                # yc = h^/2 - gpre   (3D APs: TensorScalarPtr limit)
                nc.vector.scalar_tensor_tensor(
                    yc[:, kk, :, :],
                    yh[:, t0 + 1 : t1 + 1, kk * FB : (kk + 1) * FB],
                    0.5,
                    gp[:, kk, :, :],
                    MULT, SUB,
                )
            fl = pcl.tile([128, 2, TCC, BPC], fp32, tag="fl_c")
            nc.vector.tensor_mul(fl[:], tg[:], yc[:])
            nc.vector.tensor_add(fl[:], fl[:], gp)
            nc.sync.dma_start(out_d[:, :, t0:t1, :], fl[:])

        with tc.tile_pool(name="pxg", bufs=1) as pxg:
            # xg: [:, t, 0:48] = ifg cols, [:, t, 48:64] = o cols -- both
            # flat contiguous identity-matmul rhs slices
            xgc = pxg.tile([128, T, GT * FB], b16, tag="xgc")  # 64KB/p

            # ---------------- Phase A: projections ----------------
            with (
                tc.tile_pool(name="phaseA", bufs=2) as pa,
                tc.tile_pool(name="psumA", bufs=2, space="PSUM") as psa,
            ):
                wp_sb = pa.tile([128, KT_A, NP], b16, tag="wp", bufs=1)
                nc.sync.dma_start(
                    wp_sb[:], wpt_d.rearrange("(k p) m -> p k m", p=128)
                )
                TBC = 512 // BPC   # 64 timesteps per chunk
                xgv = xgc[:, :, :].rearrange("p t (g b) -> p t g b", b=FB)
                vodd = 0
                for n in range(NCH_A):
                    xt_sb = pa.tile([128, KT_A, 512], b16, tag="xt")
                    nc.sync.dma_start(
                        xt_sb[:],
                        xt_d.rearrange("(k p) n -> p k n", p=128)[
                            :, :, n * 512 : (n + 1) * 512
                        ],
                    )
                    for m in range(MT_A):
                        ps = psa.tile([128, 512], fp32, tag="psA")
                        for k in range(KT_A):
                            nc.tensor.matmul(
                                ps[:],
                                wp_sb[:, k, m * 128 : (m + 1) * 128],
                                xt_sb[:, k, :],
                                start=(k == 0),
                                stop=(k == KT_A - 1),
                            )
                        tchunk = ps[:].rearrange("p (t b) -> p t b", b=BPC)
                        t0 = n * TBC
                        t1 = (n + 1) * TBC
                        if m < GT:
                            # round-robin DVE / ACT to keep phase A PE-bound
                            if vodd % 2 == 0:
                                nc.vector.tensor_scalar_add(
                                    xgv[:, t0:t1, m, :],
                                    tchunk,
                                    bias_sb[:, m : m + 1],
                                )
                            else:
                                nc.scalar.activation(
                                    xgv[:, t0:t1, m, :],
                                    tchunk,
                                    Identity,
                                    bias=bias_sb[:, m : m + 1],
                                )
                            vodd += 1
                        else:
                            nc.vector.tensor_scalar_add(
                                gpre[:, m - GT, t0:t1, :],
                                tchunk,
                                bias_sb[:, m : m + 1],
                            )

            # ---------------- Phase B: recurrence ----------------
            with tc.tile_pool(name="phaseB", bufs=4) as pb:
                for t in range(n_steps):
                    psi = psb.tile([128, GBI], fp32, tag="psI", name="psI",
                                   bufs=3)
                    pso = psb.tile([128, GBO], fp32, tag="psO", name="psO",
                                   bufs=3)
                    # xg(t) into both banks (clears them); no dependency on
                    # h so these prefetch during the previous step's
                    # elementwise tail
                    nc.tensor.matmul(
                        psi[:], ident_sb[:], xgc[:, t, 0:GBI],
                        start=True, stop=False,
                    )
                    nc.tensor.matmul(
                        pso[:], ident_sb[:], xgc[:, t, GBI : GBI + GBO],
                        start=True, stop=False,
                    )
                    # critical-path matmuls first: i,f,g tiles
                    for m in range(NIFG):
                        for k in range(KT_B):
                            nc.tensor.matmul(
                                psi[:, m * FB : (m + 1) * FB],
                                whh_sb[:, k, m * 128 : (m + 1) * 128],
                                yh[:, t, k * FB : (k + 1) * FB],
                                start=False,
                                stop=(m == NIFG - 1 and k == KT_B - 1),
                            )
                    # o tiles off the critical path
                    for m in range(NIFG, GT):
                        for k in range(KT_B):
                            nc.tensor.matmul(
                                pso[:, (m - NIFG) * FB : (m - NIFG + 1) * FB],
                                whh_sb[:, k, m * 128 : (m + 1) * 128],
                                yh[:, t, k * FB : (k + 1) * FB],
                                start=False,
                                stop=(m == GT - 1 and k == KT_B - 1),
                            )
                    th = pb.tile([128, GBI], fp32, tag="th", name="th")
                    nc.scalar.activation(th[:], psi[:], Tanh)
                    tho = pb.tile([128, GBO], fp32, tag="tho", name="tho")
                    nc.scalar.activation(tho[:], pso[:], Tanh)
                    # A = (th_f + 1) * c^   (= 2 sig_f c^)
                    A = pb.tile([128, KB], fp32, tag="A", name="A")
                    nc.vector.scalar_tensor_tensor(
                        A[:], th[:, 2 * FB : 4 * FB], 1.0, cst[:], ADD, MULT
                    )
                    # B = (th_i + 1) * th_g (= 2 sig_i g)
                    Bt = pb.tile([128, KB], fp32, tag="B", name="B")
                    nc.vector.scalar_tensor_tensor(
                        Bt[:], th[:, 0 : 2 * FB], 1.0, th[:, 4 * FB : 6 * FB],
                        ADD, MULT,
                    )
                    # c^' = 0.5*A + B  (= 2 c_new)
                    nc.vector.scalar_tensor_tensor(
                        cst[:], A[:], 0.5, Bt[:], MULT, ADD
                    )
                    # tau = tanh(c^' / 2) = tanh(c_new)
                    tau = pb.tile([128, KB], fp32, tag="tau", name="tau")
                    nc.scalar.activation(tau[:], cst[:], Tanh, scale=0.5)
                    # h^' = (th_o + 1) * tau (= 2 h_new)
                    nc.vector.scalar_tensor_tensor(
                        yh[:, t + 1, :], tho[:], 1.0, tau[:], ADD, MULT,
                    )
                    # interleave highway chunks so they use DVE/ACT slack
                    if (t + 1) % TCC == 0 and (t + 1) < n_steps:
                        highway_chunk((t + 1) // TCC - 1)

        # ---------------- Phase C: last highway chunk ----------------
        highway_chunk(T // TCC - 1)

    nc.compile()
    return nc


def _reverse_padded_np(x, lens):
    t = np.arange(T)
    idx = np.where(t[None, :] < lens[:, None], lens[:, None] - 1 - t[None, :], t[None, :])
    return np.take_along_axis(x, idx[:, :, None], axis=1), idx


def kernel(x, Wih_f, Whh_f, bih_f, bhh_f, Wih_b, Whh_b, bih_b, bhh_b, Wg, bg,
           x_lengths, **_unused):
    from concourse.bass_utils import run_bass_kernel_spmd

    x = np.asarray(x, dtype=np.float32)
    lens = np.asarray(x_lengths).astype(np.int64)

    xr, idx = _reverse_padded_np(x, lens)

    # tanh half-angle row scaling (torch gate order i,f,g,o):
    # i,f rows 0.5; g rows 1.0; o rows 0.5; highway rows 1.0
    rs = np.ones((NP, 1), dtype=np.float64)
    rs[0:512] = 0.5
    rs[768:1024] = 0.5

    def dir_weights(Wih, Whh, bih, bhh, wg_half, bg_half):
        Wp = np.concatenate([np.asarray(Wih), wg_half], axis=0)  # [1280, 512]
        Wp = Wp * rs
        wpt = np.ascontiguousarray(Wp.T).astype(bf16)            # [512, 1280]
        # Whh gets the row scaling AND a 0.5 for the h^ = 2h input
        Whh_s = np.asarray(Whh) * rs[0:NG] * 0.5
        whht = np.ascontiguousarray(Whh_s.T).astype(bf16)        # [256, 1024]
        bias = (np.asarray(bih) + np.asarray(bhh))
        bias = np.concatenate([bias, bg_half]) * rs[:, 0]
        return wpt, whht, bias.astype(np.float32)

    Wg = np.asarray(Wg); bg = np.asarray(bg)
    fw = dir_weights(Wih_f, Whh_f, bih_f, bhh_f, Wg[0:H], bg[0:H])
    bw = dir_weights(Wih_b, Whh_b, bih_b, bhh_b, Wg[H:2*H], bg[H:2*H])

    ident = np.eye(128, dtype=bf16)

    in_maps = []
    for c in range(NCORES):
        fwd = c < 4
        s0 = (c % 4) * BPC
        xsrc = x if fwd else xr
        xt = np.ascontiguousarray(
            xsrc[s0 : s0 + BPC].transpose(2, 1, 0).reshape(DIN, TOK)
        ).astype(bf16)
        wpt, whht, bias = fw if fwd else bw
        in_maps.append({"xt": xt, "wpt": wpt, "whht": whht, "bias": bias,
                        "ident": ident})

    if "prog" not in _PROG_CACHE:
        _PROG_CACHE["prog"] = _build_program()
    nc = _PROG_CACHE["prog"]
    _PROG_CACHE["last_inmaps"] = in_maps

    res = run_bass_kernel_spmd(nc, in_maps, core_ids=list(range(NCORES)))

    full = np.zeros((B, T, 2 * H), dtype=np.float32)
    for c in range(NCORES):
        arr = np.asarray(res.results[c]["out"], dtype=np.float32)  # [128,2,T,BPC]
        half = arr.transpose(3, 2, 1, 0).reshape(BPC, T, H)
        s0 = (c % 4) * BPC
        if c < 4:
            full[s0 : s0 + BPC, :, 0:H] = half
        else:
            # un-reverse within valid lengths
            half = np.take_along_axis(half, idx[s0 : s0 + BPC][:, :, None], axis=1)
            full[s0 : s0 + BPC, :, H : 2 * H] = half

    mask = (np.arange(T)[None, :] < lens[:, None])[:, :, None]
    full *= mask
    return full


# revision 25
# speedup vs baseline: 1.5452x; 1.0030x over previous
"""HBiLSTM Trainium2 kernel (v6c).

Strategy (8 NeuronCores):
  - cores 0-3: forward LSTM + fwd highway half, 8 samples each
  - cores 4-7: backward LSTM on host-reversed input + bwd highway half
  All cores run the SAME SPMD program; direction is encoded purely in the
  per-core input data (weights + pre-reversed/pre-transposed x).

Device layout: gate/hidden dims on SBUF partitions, batch (8) on the free
dim.  Host does all transposes / reversal / concat / masking (untimed).

The recurrence is latency-bound: total time = 512 x per-step critical
path.  v4 minimizes that path with a single chain of all 8 samples:
  - gate tile order [i,f,g,o] (natural torch order needs no permutation
    beyond i,f,g | o splitting), i/f rows pre-scaled 0.5 (tanh half-angle
    sigmoid), g rows 1.0, o rows 0.5.
  - TWO psum banks per step: ps_ifg (6 tiles) and ps_o (2 tiles).  Each
    gets its xg chunk via an identity matmul (start=True) that the Whh@h
    matmuls accumulate onto.  The identity matmuls depend only on phase A
    so they prefetch into the next buffer during the previous step's
    elementwise work; only the 12 ifg Whh matmuls + tanh_ifg sit on the
    critical path, the 4 o-tile matmuls + tanh_o run in the shadow.
  - scaled states c^ = 2c, h^ = 2h:
        A   = (th_f + 1) * c^           # 2 sig_f * c^
        B   = (th_i + 1) * th_g         # 2 sig_i * g
        c^' = 0.5*A + B                 # = 2 c_new
        tau = tanh(0.5 * c^')           # ACT free scale
        h^' = (th_o + 1) * tau          # = 2 h_new
    (scalar_tensor_tensor fusions; 0.5 for h^ as matmul input folded into
    Whh on host; output 0.5 folded into phase C.)

Phases:
  A: xg.T = Wp @ x.T + b  (Wp = [Wih(scaled); Wg_half] -> 10 tiles);
     bias-add/copy ops round-robin DVE/ACT so phase A is PE-bound.
  B: 512-step recurrence; highway chunks emitted at 128-step boundaries
     fill DVE/ACT slack.
  C: highway gate flow = g_pre + sig(g_pre) * (h^/2 - g_pre), DMA out.
"""

import numpy as np
import ml_dtypes

bf16 = ml_dtypes.bfloat16

B, T, DIN, H = 32, 512, 512, 256
NG = 4 * H          # 1024 gate rows per direction
NP = NG + H         # 1280 = gates + highway-half rows
BPC = 8             # samples per core
NCORES = 8
TOK = BPC * T       # tokens per core = 4096

_PROG_CACHE = {}


def _build_program(n_steps=T):
    import concourse.bacc as bacc
    import concourse.mybir as mybir
    import concourse.tile as tile

    fp32 = mybir.dt.float32
    b16 = mybir.dt.bfloat16
    Tanh = mybir.ActivationFunctionType.Tanh
    Sigmoid = mybir.ActivationFunctionType.Sigmoid
    Identity = mybir.ActivationFunctionType.Identity
    ADD = mybir.AluOpType.add
    MULT = mybir.AluOpType.mult
    SUB = mybir.AluOpType.subtract

    nc = bacc.Bacc(None)

    xt_d = nc.dram_tensor("xt", [DIN, TOK], b16, kind="ExternalInput")
    wpt_d = nc.dram_tensor("wpt", [DIN, NP], b16, kind="ExternalInput")
    whht_d = nc.dram_tensor("whht", [H, NG], b16, kind="ExternalInput")
    bias_d = nc.dram_tensor("bias", [NP], fp32, kind="ExternalInput")
    ident_d = nc.dram_tensor("ident", [128, 128], b16, kind="ExternalInput")
    out_d = nc.dram_tensor("out", [128, 2, T, BPC], fp32, kind="ExternalOutput")

    KT_A = DIN // 128      # 4 contraction tiles in phase A
    MT_A = NP // 128       # 10 output tiles in phase A (8 xg + 2 gpre)
    NCH_A = TOK // 512     # 8 token chunks of 512
    GT = NG // 128         # 8 gate tiles in recurrence
    NIFG = 6               # i,f,g tiles (0..5); o tiles are 6,7
    KT_B = H // 128        # 2 contraction tiles in recurrence
    FB = BPC               # 8 samples, single chain
    GBI = NIFG * FB        # 48 = ifg cols
    GBO = (GT - NIFG) * FB # 16 = o cols
    KB = KT_B * FB         # 16 = hidden cols

    with tile.TileContext(nc) as tc:
      with (
          tc.tile_pool(name="persist", bufs=1) as pp,
          tc.tile_pool(name="psumB", bufs=2, space="PSUM") as psb,
          tc.tile_pool(name="phaseC", bufs=2) as pcl,
      ):
        gpre = pp.tile([128, 2, T, BPC], fp32, tag="gpre")      # 32KB/p
        bias_sb = pp.tile([128, MT_A], fp32, tag="bias")
        nc.sync.dma_start(bias_sb[:], bias_d.rearrange("(m p) -> p m", p=128))

        whh_sb = pp.tile([128, KT_B, NG], b16, tag="whh")
        nc.sync.dma_start(whh_sb[:], whht_d.rearrange("(k p) m -> p k m", p=128))

        ident_sb = pp.tile([128, 128], b16, tag="ident")
        nc.sync.dma_start(ident_sb[:], ident_d[:, :])

        # yh layout [128, T+1, KT_B*FB]: step slice [:, t, :] is flat 2D
        yh = pp.tile([128, n_steps + 1, KB], b16, tag="yh")
        cst = pp.tile([128, KB], fp32, tag="cst")
        nc.gpsimd.memset(yh[:, 0, :], 0.0)
        nc.gpsimd.memset(cst[:], 0.0)

        def highway_chunk(t0, t1):
            w = t1 - t0
            gp = gpre[:, :, t0:t1, :]
            tg = pcl.tile([128, 2, w, BPC], fp32, tag="tg_c")
            nc.scalar.activation(tg[:], gp, Sigmoid)
            yc = pcl.tile([128, 2, w, BPC], fp32, tag="y_c")
            for kk in range(KT_B):
                # yc = h^/2 - gpre   (3D APs: TensorScalarPtr limit)
                nc.vector.scalar_tensor_tensor(
                    yc[:, kk, :, :],
                    yh[:, t0 + 1 : t1 + 1, kk * FB : (kk + 1) * FB],
                    0.5,
                    gp[:, kk, :, :],
                    MULT, SUB,
                )
            fl = pcl.tile([128, 2, w, BPC], fp32, tag="fl_c")
            nc.vector.tensor_mul(fl[:], tg[:], yc[:])
            nc.vector.tensor_add(fl[:], fl[:], gp)
            nc.sync.dma_start(out_d[:, :, t0:t1, :], fl[:])

        with tc.tile_pool(name="pxg", bufs=1) as pxg:
            # xg: [:, t, 0:48] = ifg cols, [:, t, 48:64] = o cols -- both
            # flat contiguous identity-matmul rhs slices
            xgc = pxg.tile([128, T, GT * FB], b16, tag="xgc")  # 64KB/p

            # ---------------- Phase A: projections ----------------
            with (
                tc.tile_pool(name="phaseA", bufs=2) as pa,
                tc.tile_pool(name="psumA", bufs=2, space="PSUM") as psa,
            ):
                wp_sb = pa.tile([128, KT_A, NP], b16, tag="wp", bufs=1)
                nc.sync.dma_start(
                    wp_sb[:], wpt_d.rearrange("(k p) m -> p k m", p=128)
                )
                TBC = 512 // BPC   # 64 timesteps per chunk
                xgv = xgc[:, :, :].rearrange("p t (g b) -> p t g b", b=FB)
                vodd = 0
                for n in range(NCH_A):
                    xt_sb = pa.tile([128, KT_A, 512], b16, tag="xt")
                    nc.sync.dma_start(
                        xt_sb[:],
                        xt_d.rearrange("(k p) n -> p k n", p=128)[
                            :, :, n * 512 : (n + 1) * 512
                        ],
                    )
                    for m in range(MT_A):
                        ps = psa.tile([128, 512], fp32, tag="psA")
                        for k in range(KT_A):
                            nc.tensor.matmul(
                                ps[:],
                                wp_sb[:, k, m * 128 : (m + 1) * 128],
                                xt_sb[:, k, :],
                                start=(k == 0),
                                stop=(k == KT_A - 1),
                            )
                        tchunk = ps[:].rearrange("p (t b) -> p t b", b=BPC)
                        t0 = n * TBC
                        t1 = (n + 1) * TBC
                        if m < GT:
                            # round-robin DVE / ACT to keep phase A PE-bound
                            if vodd % 2 == 0:
                                nc.vector.tensor_scalar_add(
                                    xgv[:, t0:t1, m, :],
                                    tchunk,
                                    bias_sb[:, m : m + 1],
                                )
                            else:
                                nc.scalar.activation(
                                    xgv[:, t0:t1, m, :],
                                    tchunk,
                                    Identity,
                                    bias=bias_sb[:, m : m + 1],
                                )
                            vodd += 1
                        else:
                            nc.vector.tensor_scalar_add(
                                gpre[:, m - GT, t0:t1, :],
                                tchunk,
                                bias_sb[:, m : m + 1],
                            )

            # ---------------- Phase B: recurrence ----------------
            with tc.tile_pool(name="phaseB", bufs=4) as pb:
                c_prev = cst    # zero-initialized c^ for step 0
                for t in range(n_steps):
                    psi = psb.tile([128, GBI], fp32, tag="psI", name="psI",
                                   bufs=3)
                    pso = psb.tile([128, GBO], fp32, tag="psO", name="psO",
                                   bufs=3)
                    # xg(t) into both banks (clears them); no dependency on
                    # h so these prefetch during the previous step's
                    # elementwise tail
                    nc.tensor.matmul(
                        psi[:], ident_sb[:], xgc[:, t, 0:GBI],
                        start=True, stop=False,
                    )
                    nc.tensor.matmul(
                        pso[:], ident_sb[:], xgc[:, t, GBI : GBI + GBO],
                        start=True, stop=False,
                    )
                    # critical-path matmuls first: i,f,g tiles
                    for m in range(NIFG):
                        for k in range(KT_B):
                            nc.tensor.matmul(
                                psi[:, m * FB : (m + 1) * FB],
                                whh_sb[:, k, m * 128 : (m + 1) * 128],
                                yh[:, t, k * FB : (k + 1) * FB],
                                start=False,
                                stop=(m == NIFG - 1 and k == KT_B - 1),
                            )
                    # o tiles off the critical path
                    for m in range(NIFG, GT):
                        for k in range(KT_B):
                            nc.tensor.matmul(
                                pso[:, (m - NIFG) * FB : (m - NIFG + 1) * FB],
                                whh_sb[:, k, m * 128 : (m + 1) * 128],
                                yh[:, t, k * FB : (k + 1) * FB],
                                start=False,
                                stop=(m == GT - 1 and k == KT_B - 1),
                            )
                    th = pb.tile([128, GBI], fp32, tag="th", name="th")
                    nc.scalar.activation(th[:], psi[:], Tanh)
                    tho = pb.tile([128, GBO], fp32, tag="tho", name="tho")
                    nc.scalar.activation(tho[:], pso[:], Tanh)
                    # A = (th_f + 1) * c^   (= 2 sig_f c^)
                    A = pb.tile([128, KB], fp32, tag="A", name="A")
                    nc.vector.scalar_tensor_tensor(
                        A[:], th[:, 2 * FB : 4 * FB], 1.0, c_prev[:], ADD, MULT
                    )
                    # B = (th_i + 1) * th_g (= 2 sig_i g)
                    Bt = pb.tile([128, KB], fp32, tag="B", name="B")
                    nc.vector.scalar_tensor_tensor(
                        Bt[:], th[:, 0 : 2 * FB], 1.0, th[:, 4 * FB : 6 * FB],
                        ADD, MULT,
                    )
                    # c^' = 0.5*A + B  (= 2 c_new), into a fresh tile
                    c_new = pb.tile([128, KB], fp32, tag="cn", name="cn",
                                    bufs=3)
                    nc.vector.scalar_tensor_tensor(
                        c_new[:], A[:], 0.5, Bt[:], MULT, ADD
                    )
                    c_prev = c_new
                    # tau = tanh(c^' / 2) = tanh(c_new)
                    tau = pb.tile([128, KB], fp32, tag="tau", name="tau")
                    nc.scalar.activation(tau[:], c_new[:], Tanh, scale=0.5)
                    # h^' = (th_o + 1) * tau (= 2 h_new)
                    nc.vector.scalar_tensor_tensor(
                        yh[:, t + 1, :], tho[:], 1.0, tau[:], ADD, MULT,
                    )
                    # interleave highway chunks so they use DVE/ACT slack
                    if (t + 1) == 128:
                        highway_chunk(0, 128)
                    elif (t + 1) == 256:
                        highway_chunk(128, 256)
                    elif (t + 1) == 384:
                        highway_chunk(256, 384)
                    elif (t + 1) == 480:
                        highway_chunk(384, 480)

        # ---------------- Phase C: last (small) highway chunk --------
        highway_chunk(T - 32, T)

    nc.compile()
    return nc


def _reverse_padded_np(x, lens):
    t = np.arange(T)
    idx = np.where(t[None, :] < lens[:, None], lens[:, None] - 1 - t[None, :], t[None, :])
    return np.take_along_axis(x, idx[:, :, None], axis=1), idx


def kernel(x, Wih_f, Whh_f, bih_f, bhh_f, Wih_b, Whh_b, bih_b, bhh_b, Wg, bg,
           x_lengths, **_unused):
    from concourse.bass_utils import run_bass_kernel_spmd

    x = np.asarray(x, dtype=np.float32)
    lens = np.asarray(x_lengths).astype(np.int64)

    xr, idx = _reverse_padded_np(x, lens)

    # tanh half-angle row scaling (torch gate order i,f,g,o):
    # i,f rows 0.5; g rows 1.0; o rows 0.5; highway rows 1.0
    rs = np.ones((NP, 1), dtype=np.float64)
    rs[0:512] = 0.5
    rs[768:1024] = 0.5

    def dir_weights(Wih, Whh, bih, bhh, wg_half, bg_half):
        Wp = np.concatenate([np.asarray(Wih), wg_half], axis=0)  # [1280, 512]
        Wp = Wp * rs
        wpt = np.ascontiguousarray(Wp.T).astype(bf16)            # [512, 1280]
        # Whh gets the row scaling AND a 0.5 for the h^ = 2h input
        Whh_s = np.asarray(Whh) * rs[0:NG] * 0.5
        whht = np.ascontiguousarray(Whh_s.T).astype(bf16)        # [256, 1024]
        bias = (np.asarray(bih) + np.asarray(bhh))
        bias = np.concatenate([bias, bg_half]) * rs[:, 0]
        return wpt, whht, bias.astype(np.float32)

    Wg = np.asarray(Wg); bg = np.asarray(bg)
    fw = dir_weights(Wih_f, Whh_f, bih_f, bhh_f, Wg[0:H], bg[0:H])
    bw = dir_weights(Wih_b, Whh_b, bih_b, bhh_b, Wg[H:2*H], bg[H:2*H])

    ident = np.eye(128, dtype=bf16)

    in_maps = []
    for c in range(NCORES):
        fwd = c < 4
        s0 = (c % 4) * BPC
        xsrc = x if fwd else xr
        xt = np.ascontiguousarray(
            xsrc[s0 : s0 + BPC].transpose(2, 1, 0).reshape(DIN, TOK)
        ).astype(bf16)
        wpt, whht, bias = fw if fwd else bw
        in_maps.append({"xt": xt, "wpt": wpt, "whht": whht, "bias": bias,
                        "ident": ident})

    if "prog" not in _PROG_CACHE:
        _PROG_CACHE["prog"] = _build_program()
    nc = _PROG_CACHE["prog"]
    _PROG_CACHE["last_inmaps"] = in_maps

    res = run_bass_kernel_spmd(nc, in_maps, core_ids=list(range(NCORES)))

    full = np.zeros((B, T, 2 * H), dtype=np.float32)
    for c in range(NCORES):
        arr = np.asarray(res.results[c]["out"], dtype=np.float32)  # [128,2,T,BPC]
        half = arr.transpose(3, 2, 1, 0).reshape(BPC, T, H)
        s0 = (c % 4) * BPC
        if c < 4:
            full[s0 : s0 + BPC, :, 0:H] = half
        else:
            # un-reverse within valid lengths
            half = np.take_along_axis(half, idx[s0 : s0 + BPC][:, :, None], axis=1)
            full[s0 : s0 + BPC, :, H : 2 * H] = half

    mask = (np.arange(T)[None, :] < lens[:, None])[:, :, None]
    full *= mask
    return full


# revision 28
# speedup vs baseline: 1.6151x; 1.0452x over previous
"""HBiLSTM Trainium2 kernel (v7).

Strategy (8 NeuronCores):
  - cores 0-3: forward LSTM + fwd highway half, 8 samples each
  - cores 4-7: backward LSTM on host-reversed input + bwd highway half
  All cores run the SAME SPMD program; direction is encoded purely in the
  per-core input data (weights + pre-reversed/pre-transposed x).

Device layout: gate/hidden dims on SBUF partitions, batch (8) on the free
dim.  Host does all transposes / reversal / concat / masking (untimed).

The recurrence is latency-bound: total time = 512 x per-step critical
path.  v4 minimizes that path with a single chain of all 8 samples:
  - gate tile order [i,f,g,o] (natural torch order needs no permutation
    beyond i,f,g | o splitting), i/f rows pre-scaled 0.5 (tanh half-angle
    sigmoid), g rows 1.0, o rows 0.5.
  - TWO psum banks per step: ps_ifg (6 tiles) and ps_o (2 tiles).  Each
    gets its xg chunk via an identity matmul (start=True) that the Whh@h
    matmuls accumulate onto.  The identity matmuls depend only on phase A
    so they prefetch into the next buffer during the previous step's
    elementwise work; only the 12 ifg Whh matmuls + tanh_ifg sit on the
    critical path, the 4 o-tile matmuls + tanh_o run in the shadow.
  - scaled states c^ = 2c, h^ = 2h:
        A   = (th_f + 1) * c^           # 2 sig_f * c^
        B   = (th_i + 1) * th_g         # 2 sig_i * g
        c^' = 0.5*A + B                 # = 2 c_new
        tau = tanh(0.5 * c^')           # ACT free scale
        h^' = (th_o + 1) * tau          # = 2 h_new
    (scalar_tensor_tensor fusions; 0.5 for h^ as matmul input folded into
    Whh on host; output 0.5 folded into phase C.)

Phases:
  A: xg.T = Wp @ x.T + b  (Wp = [Wih(scaled); Wg_half] -> 10 tiles);
     bias-add/copy ops round-robin DVE/ACT so phase A is PE-bound.
  B: 512-step recurrence; highway chunks emitted at 128-step boundaries
     fill DVE/ACT slack.
  C: highway gate flow = g_pre + sig(g_pre) * (h^/2 - g_pre), DMA out.
"""

import numpy as np
import ml_dtypes

bf16 = ml_dtypes.bfloat16

B, T, DIN, H = 32, 512, 512, 256
NG = 4 * H          # 1024 gate rows per direction
NP = NG + H         # 1280 = gates + highway-half rows
BPC = 8             # samples per core
NCORES = 8
TOK = BPC * T       # tokens per core = 4096

_PROG_CACHE = {}


def _build_program(n_steps=T):
    import concourse.bacc as bacc
    import concourse.mybir as mybir
    import concourse.tile as tile

    fp32 = mybir.dt.float32
    b16 = mybir.dt.bfloat16
    Tanh = mybir.ActivationFunctionType.Tanh
    Sigmoid = mybir.ActivationFunctionType.Sigmoid
    Identity = mybir.ActivationFunctionType.Identity
    ADD = mybir.AluOpType.add
    MULT = mybir.AluOpType.mult
    SUB = mybir.AluOpType.subtract

    nc = bacc.Bacc(None)

    xt_d = nc.dram_tensor("xt", [DIN, TOK], b16, kind="ExternalInput")
    wpt_d = nc.dram_tensor("wpt", [DIN, NP], b16, kind="ExternalInput")
    whht_d = nc.dram_tensor("whht", [H, NG], b16, kind="ExternalInput")
    bias_d = nc.dram_tensor("bias", [NP], fp32, kind="ExternalInput")
    ident_d = nc.dram_tensor("ident", [128, 128], b16, kind="ExternalInput")
    out_d = nc.dram_tensor("out", [128, 2, T, BPC], fp32, kind="ExternalOutput")

    KT_A = DIN // 128      # 4 contraction tiles in phase A
    MT_A = NP // 128       # 10 output tiles in phase A (8 xg + 2 gpre)
    NCH_A = TOK // 512     # 8 token chunks of 512
    GT = NG // 128         # 8 gate tiles in recurrence
    NIFG = 6               # i,f,g tiles (0..5); o tiles are 6,7
    KT_B = H // 128        # 2 contraction tiles in recurrence
    FB = BPC               # 8 samples, single chain
    GBI = NIFG * FB        # 48 = ifg cols
    GBO = (GT - NIFG) * FB # 16 = o cols
    KB = KT_B * FB         # 16 = hidden cols

    with tile.TileContext(nc) as tc:
      with (
          tc.tile_pool(name="persist", bufs=1) as pp,
          tc.tile_pool(name="psumB", bufs=2, space="PSUM") as psb,
          tc.tile_pool(name="phaseC", bufs=2) as pcl,
      ):
        gpre = pp.tile([128, 2, T, BPC], fp32, tag="gpre")      # 32KB/p
        bias_sb = pp.tile([128, MT_A], fp32, tag="bias")
        nc.sync.dma_start(bias_sb[:], bias_d.rearrange("(m p) -> p m", p=128))

        whh_sb = pp.tile([128, KT_B, NG], b16, tag="whh")
        nc.sync.dma_start(whh_sb[:], whht_d.rearrange("(k p) m -> p k m", p=128))

        ident_sb = pp.tile([128, 128], b16, tag="ident")
        nc.sync.dma_start(ident_sb[:], ident_d[:, :])

        # yh layout [128, T+1, KT_B*FB]: step slice [:, t, :] is flat 2D
        yh = pp.tile([128, n_steps + 1, KB], b16, tag="yh")
        cst = pp.tile([128, KB], fp32, tag="cst")
        nc.gpsimd.memset(yh[:, 0, :], 0.0)
        nc.gpsimd.memset(cst[:], 0.0)

        def highway_chunk(t0, t1):
            w = t1 - t0
            gp = gpre[:, :, t0:t1, :]
            tg = pcl.tile([128, 2, w, BPC], fp32, tag="tg_c")
            nc.scalar.activation(tg[:], gp, Sigmoid)
            yc = pcl.tile([128, 2, w, BPC], fp32, tag="y_c")
            for kk in range(KT_B):
                # yc = h^/2 - gpre   (3D APs: TensorScalarPtr limit)
                nc.vector.scalar_tensor_tensor(
                    yc[:, kk, :, :],
                    yh[:, t0 + 1 : t1 + 1, kk * FB : (kk + 1) * FB],
                    0.5,
                    gp[:, kk, :, :],
                    MULT, SUB,
                )
            fl = pcl.tile([128, 2, w, BPC], fp32, tag="fl_c")
            nc.vector.tensor_mul(fl[:], tg[:], yc[:])
            nc.vector.tensor_add(fl[:], fl[:], gp)
            nc.sync.dma_start(out_d[:, :, t0:t1, :], fl[:])

        with tc.tile_pool(name="pxg", bufs=1) as pxg:
            # xg: [:, t, 0:48] = ifg cols, [:, t, 48:64] = o cols -- both
            # flat contiguous identity-matmul rhs slices
            xgc = pxg.tile([128, T, GT * FB], b16, tag="xgc")  # 64KB/p

            # ---------------- Phase A: projections ----------------
            with (
                tc.tile_pool(name="phaseA", bufs=2) as pa,
                tc.tile_pool(name="psumA", bufs=2, space="PSUM") as psa,
            ):
                wp_sb = pa.tile([128, KT_A, NP], b16, tag="wp", bufs=1)
                nc.sync.dma_start(
                    wp_sb[:], wpt_d.rearrange("(k p) m -> p k m", p=128)
                )
                TBC = 512 // BPC   # 64 timesteps per chunk
                xgv = xgc[:, :, :].rearrange("p t (g b) -> p t g b", b=FB)
                vodd = 0
                for n in range(NCH_A):
                    xt_sb = pa.tile([128, KT_A, 512], b16, tag="xt")
                    nc.sync.dma_start(
                        xt_sb[:],
                        xt_d.rearrange("(k p) n -> p k n", p=128)[
                            :, :, n * 512 : (n + 1) * 512
                        ],
                    )
                    for m in range(MT_A):
                        ps = psa.tile([128, 512], fp32, tag="psA")
                        for k in range(KT_A):
                            nc.tensor.matmul(
                                ps[:],
                                wp_sb[:, k, m * 128 : (m + 1) * 128],
                                xt_sb[:, k, :],
                                start=(k == 0),
                                stop=(k == KT_A - 1),
                            )
                        tchunk = ps[:].rearrange("p (t b) -> p t b", b=BPC)
                        t0 = n * TBC
                        t1 = (n + 1) * TBC
                        if m < GT:
                            # round-robin DVE / ACT to keep phase A PE-bound
                            if vodd % 2 == 0:
                                nc.vector.tensor_scalar_add(
                                    xgv[:, t0:t1, m, :],
                                    tchunk,
                                    bias_sb[:, m : m + 1],
                                )
                            else:
                                nc.scalar.activation(
                                    xgv[:, t0:t1, m, :],
                                    tchunk,
                                    Identity,
                                    bias=bias_sb[:, m : m + 1],
                                )
                            vodd += 1
                        else:
                            nc.vector.tensor_scalar_add(
                                gpre[:, m - GT, t0:t1, :],
                                tchunk,
                                bias_sb[:, m : m + 1],
                            )

            # ---------------- Phase B: recurrence ----------------
            with tc.tile_pool(name="phaseB", bufs=4) as pb:
                c_prev = cst    # zero-initialized c^ for step 0
                for t in range(n_steps):
                    # THREE psum banks: f (tiles 0,1) | i,g (2..5) | o (6,7)
                    # in the [f,i,g,o] permuted tile order.  tanh_f fires
                    # after only 4 Whh matmuls so A starts early; tanh_ig
                    # and tanh_o run in its shadow.
                    psf = psb.tile([128, 2 * FB], fp32, tag="psF", name="psF")
                    psig = psb.tile([128, 4 * FB], fp32, tag="psG", name="psG")
                    pso = psb.tile([128, GBO], fp32, tag="psO", name="psO")
                    # xg(t) into the banks (clears them); no dependency on
                    # h so these prefetch during the previous step's
                    # elementwise tail
                    nc.tensor.matmul(
                        psf[:], ident_sb[:], xgc[:, t, 0 : 2 * FB],
                        start=True, stop=False,
                    )
                    nc.tensor.matmul(
                        psig[:], ident_sb[:], xgc[:, t, 2 * FB : 6 * FB],
                        start=True, stop=False,
                    )
                    nc.tensor.matmul(
                        pso[:], ident_sb[:], xgc[:, t, GBI : GBI + GBO],
                        start=True, stop=False,
                    )
                    # critical-path matmuls first: f tiles, then i,g, then o
                    for m in range(2):
                        for k in range(KT_B):
                            nc.tensor.matmul(
                                psf[:, m * FB : (m + 1) * FB],
                                whh_sb[:, k, m * 128 : (m + 1) * 128],
                                yh[:, t, k * FB : (k + 1) * FB],
                                start=False,
                                stop=(m == 1 and k == KT_B - 1),
                            )
                    for m in range(2, NIFG):
                        for k in range(KT_B):
                            nc.tensor.matmul(
                                psig[:, (m - 2) * FB : (m - 1) * FB],
                                whh_sb[:, k, m * 128 : (m + 1) * 128],
                                yh[:, t, k * FB : (k + 1) * FB],
                                start=False,
                                stop=(m == NIFG - 1 and k == KT_B - 1),
                            )
                    for m in range(NIFG, GT):
                        for k in range(KT_B):
                            nc.tensor.matmul(
                                pso[:, (m - NIFG) * FB : (m - NIFG + 1) * FB],
                                whh_sb[:, k, m * 128 : (m + 1) * 128],
                                yh[:, t, k * FB : (k + 1) * FB],
                                start=False,
                                stop=(m == GT - 1 and k == KT_B - 1),
                            )
                    thf = pb.tile([128, KB], fp32, tag="thf", name="thf")
                    nc.scalar.activation(thf[:], psf[:], Tanh)
                    thig = pb.tile([128, 4 * FB], fp32, tag="thig", name="thig")
                    nc.scalar.activation(thig[:], psig[:], Tanh)
                    tho = pb.tile([128, GBO], fp32, tag="tho", name="tho")
                    nc.scalar.activation(tho[:], pso[:], Tanh)
                    # A = (th_f + 1) * c^   (= 2 sig_f c^)
                    A = pb.tile([128, KB], fp32, tag="A", name="A")
                    nc.vector.scalar_tensor_tensor(
                        A[:], thf[:], 1.0, c_prev[:], ADD, MULT
                    )
                    # B = (th_i + 1) * th_g (= 2 sig_i g)
                    Bt = pb.tile([128, KB], fp32, tag="B", name="B")
                    nc.vector.scalar_tensor_tensor(
                        Bt[:], thig[:, 0 : 2 * FB], 1.0,
                        thig[:, 2 * FB : 4 * FB], ADD, MULT,
                    )
                    # c^' = 0.5*A + B  (= 2 c_new), into a fresh tile
                    c_new = pb.tile([128, KB], fp32, tag="cn", name="cn",
                                    bufs=3)
                    nc.vector.scalar_tensor_tensor(
                        c_new[:], A[:], 0.5, Bt[:], MULT, ADD
                    )
                    c_prev = c_new
                    # tau = tanh(c^' / 2) = tanh(c_new)
                    tau = pb.tile([128, KB], fp32, tag="tau", name="tau")
                    nc.scalar.activation(tau[:], c_new[:], Tanh, scale=0.5)
                    # h^' = (th_o + 1) * tau (= 2 h_new)
                    nc.vector.scalar_tensor_tensor(
                        yh[:, t + 1, :], tho[:], 1.0, tau[:], ADD, MULT,
                    )
                    # interleave highway chunks so they use DVE/ACT slack
                    if (t + 1) == 128:
                        highway_chunk(0, 128)
                    elif (t + 1) == 256:
                        highway_chunk(128, 256)
                    elif (t + 1) == 384:
                        highway_chunk(256, 384)
                    elif (t + 1) == 480:
                        highway_chunk(384, 480)

        # ---------------- Phase C: last (small) highway chunk --------
        highway_chunk(T - 32, T)

    nc.compile()
    return nc


def _reverse_padded_np(x, lens):
    t = np.arange(T)
    idx = np.where(t[None, :] < lens[:, None], lens[:, None] - 1 - t[None, :], t[None, :])
    return np.take_along_axis(x, idx[:, :, None], axis=1), idx


def kernel(x, Wih_f, Whh_f, bih_f, bhh_f, Wih_b, Whh_b, bih_b, bhh_b, Wg, bg,
           x_lengths, **_unused):
    from concourse.bass_utils import run_bass_kernel_spmd

    x = np.asarray(x, dtype=np.float32)
    lens = np.asarray(x_lengths).astype(np.int64)

    xr, idx = _reverse_padded_np(x, lens)

    # gate reorder torch [i,f,g,o] -> device [f,i,g,o] (f first so tanh_f
    # fires after only 4 Whh matmuls)
    perm = np.concatenate([np.arange(256, 512), np.arange(0, 256),
                           np.arange(512, 768), np.arange(768, 1024)])
    # tanh half-angle row scaling (device order f,i,g,o):
    # f,i rows 0.5; g rows 1.0; o rows 0.5; highway rows 1.0
    rs = np.ones((NP, 1), dtype=np.float64)
    rs[0:512] = 0.5
    rs[768:1024] = 0.5

    def dir_weights(Wih, Whh, bih, bhh, wg_half, bg_half):
        Wp = np.concatenate([np.asarray(Wih)[perm], wg_half], axis=0)
        Wp = Wp * rs                                             # [1280, 512]
        wpt = np.ascontiguousarray(Wp.T).astype(bf16)            # [512, 1280]
        # Whh gets the row scaling AND a 0.5 for the h^ = 2h input
        Whh_s = np.asarray(Whh)[perm] * rs[0:NG] * 0.5
        whht = np.ascontiguousarray(Whh_s.T).astype(bf16)        # [256, 1024]
        bias = (np.asarray(bih) + np.asarray(bhh))[perm]
        bias = np.concatenate([bias, bg_half]) * rs[:, 0]
        return wpt, whht, bias.astype(np.float32)

    Wg = np.asarray(Wg); bg = np.asarray(bg)
    fw = dir_weights(Wih_f, Whh_f, bih_f, bhh_f, Wg[0:H], bg[0:H])
    bw = dir_weights(Wih_b, Whh_b, bih_b, bhh_b, Wg[H:2*H], bg[H:2*H])

    ident = np.eye(128, dtype=bf16)

    in_maps = []
    for c in range(NCORES):
        fwd = c < 4
        s0 = (c % 4) * BPC
        xsrc = x if fwd else xr
        xt = np.ascontiguousarray(
            xsrc[s0 : s0 + BPC].transpose(2, 1, 0).reshape(DIN, TOK)
        ).astype(bf16)
        wpt, whht, bias = fw if fwd else bw
        in_maps.append({"xt": xt, "wpt": wpt, "whht": whht, "bias": bias,
                        "ident": ident})

    if "prog" not in _PROG_CACHE:
        _PROG_CACHE["prog"] = _build_program()
    nc = _PROG_CACHE["prog"]
    _PROG_CACHE["last_inmaps"] = in_maps

    res = run_bass_kernel_spmd(nc, in_maps, core_ids=list(range(NCORES)))

    full = np.zeros((B, T, 2 * H), dtype=np.float32)
    for c in range(NCORES):
        arr = np.asarray(res.results[c]["out"], dtype=np.float32)  # [128,2,T,BPC]
        half = arr.transpose(3, 2, 1, 0).reshape(BPC, T, H)
        s0 = (c % 4) * BPC
        if c < 4:
            full[s0 : s0 + BPC, :, 0:H] = half
        else:
            # un-reverse within valid lengths
            half = np.take_along_axis(half, idx[s0 : s0 + BPC][:, :, None], axis=1)
            full[s0 : s0 + BPC, :, H : 2 * H] = half

    mask = (np.arange(T)[None, :] < lens[:, None])[:, :, None]
    full *= mask
    return full
